# revision 7
# baseline (speedup 1.0000x reference)
"""Contrastive-loss kernel for trn2 (8 NeuronCores, SPMD).

The reference loss reduces to a Gram matrix G = F.T @ F over the
flattened input F [N=524288, T=64] (128 MiB fp32), followed by a tiny
[64,64] masked margin reduction.  Each core streams a 16 MiB row-shard
of F through SBUF, casting fp32->bf16 inline in the SWDGE DMA, and
accumulates chunk.T @ chunk matmuls into one PSUM accumulator (fp32).
The 8 partial [64,64] Grams are summed on the host, where the masked
margin reduction (negligible work) also runs.

Perf notes (from NTFF trace analysis):
  - The DMA stream (~425 GB/s per-NC HBM fair share) is the bottleneck;
    NRT pre/postamble (~6us + ~8us incl. the 255-semaphore sweep) is
    fixed per-execution overhead.
  - On this platform the even-numbered cores each have one SDMA edge
    engine (engine 0 or engine 15, persistent per core) that runs
    ~1.2-1.35x slower than the other fifteen, which turned the worst
    core's stream from ~42.5us into ~49-51.5us.  Mitigation: engines 0
    and 15 get ~24% less data than the other 14.  Engine k serves a
    fixed set of 8 partitions (0:{0-3,32-35}, 15:{92-95,124-127}), so
    the load is shaped per-partition:
      * base stream: all 128 partitions get B=400 rows (tiles of 8192
        rows -> 64 rows/partition).
      * extra stream: partitions {4-31, 36-91, 96-123} (exactly the 8
        partitions of each of the 14 fast engines) get M=128 more rows
        via three DMAs; partitions 0-3/32-35/92-95 of the extra tile
        are zeroed (DVE memset) and contribute nothing to the Gram.
        The extra chunks are consumed as K=124 matmuls (PE requires
        base partition in {0,32,64}; zero rows are free).
  - Tail: DVE combines the two diagonal PSUM blocks, sync(HWDGE) stores
    the [64,64] result, gpsimd waits for store completion then does a
    narrow dma_reset (keeps the bacc end-of-block DRAIN at ~50ns
    instead of ~1.3us of ring-quiesce work).  Manual sem clears are
    omitted: NRT's postamble sweep resets all user semaphores anyway.
"""

import contextlib

import numpy as np

import concourse.bacc as bacc
import concourse.mybir as mybir
from concourse.bass_utils import run_bass_kernel_spmd

MARGIN = 60000.0
S = 64                      # time steps (Gram dim)
N_TOTAL = 2 * 8 * 32 * 32 * 32   # 524288 flattened rows
N_CORES = 8
N_SHARD = N_TOTAL // N_CORES     # 65536 rows per core
P = 128                     # SBUF partitions

B = 400                     # base rows per partition (all 128 partitions)
M = 128                     # extra rows per fast partition (112 of them)
BASE_ROWS = P * B           # 51200
BASE_TILE_ROWS = [8192] * 6 + [2048]
assert sum(BASE_TILE_ROWS) == BASE_ROWS
# extra stream: (partition range, #partitions) -> M rows each
EXTRA_RANGES = [(4, 28), (36, 56), (96, 28)]
assert BASE_ROWS + sum(n for _, n in EXTRA_RANGES) * M == N_SHARD
KX = 124                    # extra-chunk contraction depth (zeros padded)

_CACHE = {}
LAST_RESULTS = None         # BassKernelResults of the most recent run


def _build_nc():
    nc = bacc.Bacc("TRN2", target_bir_lowering=False, debug=False,
                   num_devices=N_CORES)
    x = nc.dram_tensor("x", [N_SHARD, S], mybir.dt.float32,
                       kind="ExternalInput")
    g = nc.dram_tensor("g", [S, S], mybir.dt.float32, kind="ExternalOutput")

    # base-tile DRAM views: tile of R rows -> partition p holds R/128
    # consecutive rows (one contiguous 256*(R/128)-byte descriptor line).
    base_views = []
    row0 = 0
    for R in BASE_TILE_ROWS:
        base_views.append(x[row0:row0 + R, :].rearrange(
            "(p r) c -> p (r c)", p=P, r=R // P))
        row0 += R
    # extra DRAM views: np partitions x M contiguous rows each.
    extra_views = []
    for _, npart in EXTRA_RANGES:
        R = npart * M
        extra_views.append(x[row0:row0 + R, :].rearrange(
            "(p r) c -> p (r c)", p=npart, r=M))
        row0 += R
    assert row0 == N_SHARD

    base_free = [(R // P) * S for R in BASE_TILE_ROWS]
    boffs = [0]
    for fe in base_free:
        boffs.append(boffs[-1] + fe)
    xfree = M * S               # extra tile free elems per partition

    n_base = len(BASE_TILE_ROWS)
    n_extra = len(EXTRA_RANGES)

    with (
        nc.sbuf_tensor("xbuf", [P, boffs[-1]], mybir.dt.bfloat16) as xbuf,
        nc.sbuf_tensor("xext", [P, xfree], mybir.dt.bfloat16) as xext,
        nc.psum_tensor("acc", [2 * S, 2 * S], mybir.dt.float32) as acc,
        nc.sbuf_tensor("obuf", [S, S], mybir.dt.float32) as obuf,
        nc.semaphore("z_sem") as z_sem,
        nc.semaphore("pe_sem") as pe_sem,
        nc.semaphore("out_sem") as out_sem,
        nc.semaphore("fin_sem") as fin_sem,
        contextlib.ExitStack() as stack,
    ):
        bsems = [stack.enter_context(nc.semaphore(f"bsem{k}"))
                 for k in range(n_base)]
        xsems = [stack.enter_context(nc.semaphore(f"xsem{k}"))
                 for k in range(n_extra)]

        with nc.Block() as block:

            @block.gpsimd
            def _(gp):
                # base tile 0 first so engines 0/15 start streaming at once
                gp.dma_start(xbuf[:, boffs[0]:boffs[1]],
                             base_views[0]).then_inc(bsems[0], 16)
                # extra stream (fast engines only); wait for the zero-fill
                # of the pad partitions so the DMA writes can't race it.
                gp.wait_ge(z_sem, 1)
                for k, (p0, npart) in enumerate(EXTRA_RANGES):
                    gp.dma_start(xext[p0:p0 + npart, :],
                                 extra_views[k]).then_inc(xsems[k], 16)
                for k in range(1, n_base):
                    gp.dma_start(xbuf[:, boffs[k]:boffs[k + 1]],
                                 base_views[k]).then_inc(bsems[k], 16)
                # keep this engine stream alive until the output store has
                # landed in HBM (NRT's postamble then resets all sems).
                gp.wait_ge(fin_sem, 16)
                # narrow ring-state reset here keeps the bacc block-end
                # DRAIN short (~50ns instead of ~1.3us of quiesce work).
                gp.dma_reset()

            @block.vector
            def _(v):
                # zero the whole extra tile (pad partitions stay zero; the
                # data partitions are overwritten by the extra DMAs).
                v.memset(xext[:], 0.0).then_inc(z_sem, 1)
                v.wait_ge(pe_sem, 1)
                v.tensor_copy(obuf[:], acc[:S, :S])
                v.tensor_add(obuf[:], obuf[:],
                             acc[S:, S:]).then_inc(out_sem, 1)

            @block.tensor
            def _(te):
                # Pack 2 row-chunks per matmul: lhsT = rhs = [A|B],
                # accumulating [[A'A, A'B], [B'A, B'B]]; the two diagonal
                # 64x64 blocks sum to the Gram contribution.
                total_mm = sum(fe // (2 * S) for fe in base_free) \
                    + xfree // (2 * S)
                n = 0

                def mm(c):
                    nonlocal n
                    r = te.matmul(acc[:], c, c, start=(n == 0),
                                  stop=(n == total_mm - 1))
                    n += 1
                    if n == total_mm:
                        r.then_inc(pe_sem, 1)

                te.wait_ge(bsems[0], 16)
                for j in range(base_free[0] // (2 * S)):
                    mm(xbuf[:, j * 2 * S:(j + 1) * 2 * S])
                for sem in xsems:
                    te.wait_ge(sem, 16)
                for j in range(xfree // (2 * S)):
                    mm(xext[0:KX, j * 2 * S:(j + 1) * 2 * S])
                for k in range(1, n_base):
                    te.wait_ge(bsems[k], 16)
                    for j in range(base_free[k] // (2 * S)):
                        mm(xbuf[:, boffs[k] + j * 2 * S:
                                boffs[k] + (j + 1) * 2 * S])

            @block.sync
            def _(sy):
                sy.wait_ge(out_sem, 1)
                sy.dma_start(g[:], obuf[:]).then_inc(fin_sem, 16)

    nc.compile()
    return nc


def get_nc():
    if "nc" not in _CACHE:
        _CACHE["nc"] = _build_nc()
    return _CACHE["nc"]


def _device_partial_grams(flat: np.ndarray, **run_kwargs) -> np.ndarray:
    """Run the SPMD bass kernel; return the 8 partial Grams [8, 64, 64]."""
    global LAST_RESULTS
    nc = get_nc()
    in_maps = [
        {"x": flat[c * N_SHARD:(c + 1) * N_SHARD]} for c in range(N_CORES)
    ]
    LAST_RESULTS = run_bass_kernel_spmd(
        nc, in_maps, core_ids=list(range(N_CORES)), **run_kwargs
    )
    return np.stack([LAST_RESULTS.results[c]["g"] for c in range(N_CORES)])


def kernel(input: np.ndarray, **run_kwargs) -> np.ndarray:
    flat = np.ascontiguousarray(
        np.asarray(input, dtype=np.float32).reshape(N_TOTAL, S)
    )
    partials = _device_partial_grams(flat, **run_kwargs)

    gram = partials.astype(np.float64).sum(axis=0)
    sq = np.diag(gram)
    dist = sq[:, None] + sq[None, :] - 2.0 * gram
    idx = np.arange(S)
    lower = idx[:, None] > idx[None, :]
    adjacent = (idx[:, None] - idx[None, :]) == 1
    per_pair = np.where(adjacent, np.maximum(0.0, MARGIN - dist), dist)
    loss = np.where(lower, per_pair, 0.0).sum() / (S * (S - 1) * 1000)
    return np.asarray(loss, dtype=np.float32)


# revision 11
# speedup vs baseline: 1.0414x; 1.0414x over previous
"""Contrastive-loss kernel for trn2 (8 NeuronCores, SPMD).

The reference loss reduces to a Gram matrix G = F.T @ F over the
flattened input F [N=524288, T=64] (128 MiB fp32), followed by a tiny
[64,64] masked margin reduction.  Each core streams a 16 MiB row-shard
of F through SBUF, casting fp32->bf16 inline in the SWDGE DMA, and
accumulates chunk.T @ chunk matmuls into one PSUM accumulator (fp32).
The 8 partial [64,64] Grams are summed on the host, where the masked
margin reduction (negligible work) also runs.

Perf notes (from NTFF trace analysis + DMA probes):
  - The DMA stream (~425 GB/s per-NC fair share) is the bottleneck; NRT
    pre/postamble (~6us + ~8us incl. the 255-semaphore sweep) is fixed.
  - SWDGE deals a DMA's per-partition-line descriptors round-robin
    across the 16 SDMA engines in emission order, cumulatively across
    DMAs (measured: a k-descriptor DMA advances the deal by k).  With
    128-descriptor tiles the deal stays aligned: partition p <-> engine
    p mod 16.
  - On this platform every even core has one persistently slow edge
    engine (engine 0 on some cores, engine 15 on others, ~1.2-1.35x),
    which stretched the worst core's stream from ~42.5us to ~49-51.5us.
    Mitigation: engines 0 and 15 get ~24% less data.  Base stream: all
    128 partitions x B=400 rows (uniform).  Extra stream: 8 DMAs of 14
    descriptors each, steered onto engines 1..14 by interposing tiny
    1-2 descriptor dummy DMAs that eat the engine-0/15 deal slots; the
    extra tile is contiguous partitions [0,112) x M=128 rows, consumed
    as K=112 matmuls (PE base partition must be in {0,32,64}).
  - Tail: DVE combines the two diagonal PSUM blocks, sync(HWDGE) stores
    the [64,64] result, gpsimd waits for store completion then does a
    narrow dma_reset (keeps the bacc end-of-block DRAIN at ~50ns).
    Manual sem clears are omitted: NRT's postamble sweep resets all
    user semaphores every execution anyway.
"""

import contextlib

import numpy as np

import concourse.bacc as bacc
import concourse.mybir as mybir
from concourse.bass_utils import run_bass_kernel_spmd

MARGIN = 60000.0
S = 64                      # time steps (Gram dim)
N_TOTAL = 2 * 8 * 32 * 32 * 32   # 524288 flattened rows
N_CORES = 8
N_SHARD = N_TOTAL // N_CORES     # 65536 rows per core
P = 128                     # SBUF partitions

B = 400                     # base rows per partition (128 partitions)
M = 128                     # extra rows per partition on [0,112)
KX = 112                    # extra-tile partitions / contraction depth
NXD = 8                     # extra DMAs, 14 descriptors each
BASE_ROWS = P * B           # 51200
BASE_TILE_ROWS = [8192] * 6 + [2048]
assert sum(BASE_TILE_ROWS) == BASE_ROWS
assert BASE_ROWS + KX * M == N_SHARD
assert NXD * 14 == KX

_CACHE = {}
LAST_RESULTS = None         # BassKernelResults of the most recent run


def _build_nc():
    nc = bacc.Bacc("TRN2", target_bir_lowering=False, debug=False,
                   num_devices=N_CORES)
    x = nc.dram_tensor("x", [N_SHARD, S], mybir.dt.float32,
                       kind="ExternalInput")
    g = nc.dram_tensor("g", [S, S], mybir.dt.float32, kind="ExternalOutput")

    # base-tile DRAM views: tile of R rows -> partition p holds R/128
    # consecutive rows (one contiguous 256*(R/128)-byte descriptor line).
    base_views = []
    row0 = 0
    for R in BASE_TILE_ROWS:
        base_views.append(x[row0:row0 + R, :].rearrange(
            "(p r) c -> p (r c)", p=P, r=R // P))
        row0 += R
    # extra DRAM views: 14 partitions x M contiguous rows each.
    extra_views = []
    for _ in range(NXD):
        R = 14 * M
        extra_views.append(x[row0:row0 + R, :].rearrange(
            "(p r) c -> p (r c)", p=14, r=M))
        row0 += R
    assert row0 == N_SHARD

    base_free = [(R // P) * S for R in BASE_TILE_ROWS]
    boffs = [0]
    for fe in base_free:
        boffs.append(boffs[-1] + fe)
    xfree = M * S               # extra tile free elems per partition

    n_base = len(BASE_TILE_ROWS)

    with (
        nc.sbuf_tensor("xbuf", [P, boffs[-1]], mybir.dt.bfloat16) as xbuf,
        nc.sbuf_tensor("xext", [P, xfree], mybir.dt.bfloat16) as xext,
        nc.sbuf_tensor("scrap", [17, 2 * S], mybir.dt.bfloat16) as scrap,
        nc.psum_tensor("acc", [2 * S, 2 * S], mybir.dt.float32) as acc,
        nc.sbuf_tensor("obuf", [S, S], mybir.dt.float32) as obuf,
        nc.semaphore("xsem") as xsem,
        nc.semaphore("junk_sem") as junk_sem,
        nc.semaphore("pe_sem") as pe_sem,
        nc.semaphore("out_sem") as out_sem,
        nc.semaphore("fin_sem") as fin_sem,
        contextlib.ExitStack() as stack,
    ):
        bsems = [stack.enter_context(nc.semaphore(f"bsem{k}"))
                 for k in range(n_base)]

        with nc.Block() as block:

            @block.gpsimd
            def _(gp):
                def dummy(n):
                    # n-descriptor filler DMA: advances the SWDGE engine
                    # deal by n so extra DMAs skip engines 0/15.  Junk
                    # bytes (n x 512B) -> scrap; nothing waits on it.
                    # n=17 advances the deal by 1 net (a [1,x] dest would
                    # be sprayed across all 16 engines, so 17 partitions
                    # stand in for +1); n=2 jumps the 15->0 boundary.
                    gp.dma_start(scrap[0:n, :],
                                 x[0:2 * n, :].rearrange(
                                     "(p r) c -> p (r c)", p=n, r=2)
                                 ).then_inc(junk_sem, 16)

                # base tiles 0,1 first: engines 0/15 stream continuously
                # and PE has work while the extra stream is set up.
                for k in (0, 1):
                    gp.dma_start(xbuf[:, boffs[k]:boffs[k + 1]],
                                 base_views[k]).then_inc(bsems[k], 16)
                # extra stream on engines 1..14 only (deal offset: +17,
                # then per extra 14 data descs + 2 across the 0/15
                # boundary; final +17 makes the whole execution advance
                # the deal by 0 mod 16 so re-executions stay aligned).
                dummy(17)
                for j in range(NXD):
                    gp.dma_start(xext[14 * j:14 * (j + 1), :],
                                 extra_views[j]).then_inc(xsem, 16)
                    dummy(2) if j < NXD - 1 else dummy(17)
                for k in range(2, n_base):
                    gp.dma_start(xbuf[:, boffs[k]:boffs[k + 1]],
                                 base_views[k]).then_inc(bsems[k], 16)
                # keep this engine stream alive until the output store has
                # landed in HBM (NRT's postamble then resets all sems).
                gp.wait_ge(fin_sem, 16)
                gp.dma_reset()

            @block.tensor
            def _(te):
                # Pack 2 row-chunks per matmul: lhsT = rhs = [A|B],
                # accumulating [[A'A, A'B], [B'A, B'B]]; the two diagonal
                # 64x64 blocks sum to the Gram contribution.
                total_mm = sum(fe // (2 * S) for fe in base_free) \
                    + xfree // (2 * S)
                n = 0

                def mm(c):
                    nonlocal n
                    r = te.matmul(acc[:], c, c, start=(n == 0),
                                  stop=(n == total_mm - 1))
                    n += 1
                    if n == total_mm:
                        r.then_inc(pe_sem, 1)

                for k in (0, 1):
                    te.wait_ge(bsems[k], 16)
                    for j in range(base_free[k] // (2 * S)):
                        mm(xbuf[:, boffs[k] + j * 2 * S:
                                boffs[k] + (j + 1) * 2 * S])
                te.wait_ge(xsem, 16 * NXD)
                for j in range(xfree // (2 * S)):
                    mm(xext[0:KX, j * 2 * S:(j + 1) * 2 * S])
                for k in range(2, n_base):
                    te.wait_ge(bsems[k], 16)
                    for j in range(base_free[k] // (2 * S)):
                        mm(xbuf[:, boffs[k] + j * 2 * S:
                                boffs[k] + (j + 1) * 2 * S])

            @block.vector
            def _(v):
                v.wait_ge(pe_sem, 1)
                v.tensor_copy(obuf[:], acc[:S, :S])
                v.tensor_add(obuf[:], obuf[:],
                             acc[S:, S:]).then_inc(out_sem, 1)

            @block.sync
            def _(sy):
                sy.wait_ge(out_sem, 1)
                sy.dma_start(g[:], obuf[:]).then_inc(fin_sem, 16)

    nc.compile()
    return nc


def get_nc():
    if "nc" not in _CACHE:
        _CACHE["nc"] = _build_nc()
    return _CACHE["nc"]


def _device_partial_grams(flat: np.ndarray, **run_kwargs) -> np.ndarray:
    """Run the SPMD bass kernel; return the 8 partial Grams [8, 64, 64]."""
    global LAST_RESULTS
    nc = get_nc()
    in_maps = [
        {"x": flat[c * N_SHARD:(c + 1) * N_SHARD]} for c in range(N_CORES)
    ]
    LAST_RESULTS = run_bass_kernel_spmd(
        nc, in_maps, core_ids=list(range(N_CORES)), **run_kwargs
    )
    return np.stack([LAST_RESULTS.results[c]["g"] for c in range(N_CORES)])


def kernel(input: np.ndarray, **run_kwargs) -> np.ndarray:
    flat = np.ascontiguousarray(
        np.asarray(input, dtype=np.float32).reshape(N_TOTAL, S)
    )
    partials = _device_partial_grams(flat, **run_kwargs)

    gram = partials.astype(np.float64).sum(axis=0)
    sq = np.diag(gram)
    dist = sq[:, None] + sq[None, :] - 2.0 * gram
    idx = np.arange(S)
    lower = idx[:, None] > idx[None, :]
    adjacent = (idx[:, None] - idx[None, :]) == 1
    per_pair = np.where(adjacent, np.maximum(0.0, MARGIN - dist), dist)
    loss = np.where(lower, per_pair, 0.0).sum() / (S * (S - 1) * 1000)
    return np.asarray(loss, dtype=np.float32)
